# revision 24
# baseline (speedup 1.0000x reference)
"""Exact attention (B=2, N=2048, H=16, D=64, fp32) on 8 Trainium2 NeuronCores.

v6 design, built from HW microbenchmarks (mb.py). Measured facts driving it:
  - ACT exp runs at ~1.23 ns/elem (not the documented 1 cyc @1.2GHz), so
    exp over all N^2 elems is a ~170us/core floor: ACT paces the kernel.
  - Matmuls with K=64 contraction stream the moving operand at ~half the
    K=128 rate (~573 vs ~330 ns per 512-col MM) regardless of dtype, and
    every matmul pays its LDWEIGHTS serially. The old baseline (f32r S +
    P^T-stationary PV with 128-col LDW churn) was PE-bound at ~272us busy.
  - Fixes here: Q/K in fp16, with K zero-padded per head to a full 128
    partitions (the zero rows contribute exactly 0, so the S matmul runs
    at the K=128 moving rate: ~84us instead of ~147us), PV in O^T
    orientation with V' = [V | 1] bf16 stationary (65-col LDW, moving P^T
    at K=128 rate: ~85us), deep pt buffering (bufs=14) so ACT runs far
    ahead of the PV consumers. PE ends up fully hidden under ACT.
  - Finalize: device returns unnormalized O^T [65, N] per head (row 64 =
    softmax denominator from the V' ones column) straight to DRAM; host
    divides and transposes during unshard (O(N*d) postprocess, same bytes
    DMA'd as the normalized [N, 64] output would be).

Sharding: 32 (batch, head) pairs across 8 cores, 4 heads per core; Q^T/K^T
stored packed 2-heads-per-128-partitions (d=64 rows each), heads processed
one chunk at a time.

Per-core schedule, per (n-chunk of 1024, pair, head), m-blocks of 128:
  PSUM: S ring of 3 [128,1024] f32 (6 banks) + one OT [65,2,512] (2 banks).
  per mb: S(mb) [2 MMs], PV(mb-LAG) [2 MMs accumulating into OT],
  exp(mb) [one ACT instr, S->pt bf16]. S runs ahead inside the exp
  window so the S->exp handoff semaphore never lands on the ACT critical
  path; PV lags by LAG windows and fills PE gaps.
  Chunk end: DVE evacuates OT to SBUF fp32, DMA out [65, 1024]; the next
  chunk's first PV (start=True) waits on that evac (Tile serializes the
  PSUM bank reuse), which is hidden since it happens LAG windows in.
"""

import os
import sys

os.environ.setdefault("MYCRO_LOCAL_CACHE", "1")
sys.path.insert(0, "/opt/trn_rl_repo")

import numpy as np

import concourse.bacc as bacc
import concourse.mybir as mybir
import concourse.tile as tile
from concourse.bass_utils import run_bass_kernel_spmd

f32 = mybir.dt.float32
f16 = mybir.dt.float16
bf16 = mybir.dt.bfloat16

B, N, H, D = 2, 2048, 16, 64
HEADS_PER_CORE = 4
N_CORES = 8
NH = 1024          # n-chunk width
N_MB = N // 128    # 16 m-blocks of 128 rows
DV = D + 1         # V plus ones column
EXP = mybir.ActivationFunctionType.Exp


def emit_body(nc, qT, kT, vp, out, pools):
    qk_p, vt_p, spool, ppool, opool, finsb = pools

    # --- input DMAs ---
    qts, kts, vts = [], {}, []
    for pair in range(2):
        qt = qk_p.tile([128, N], f16, name=f"qt_{pair}", tag=f"qt{pair}")
        nc.sync.dma_start(out=qt, in_=qT[pair])
        qts.append(qt)
        for i in range(2):
            hh = 2 * pair + i
            # K for head i zero-padded to 128 rows (data at rows 64i..64i+63,
            # zeros elsewhere): S matmul contracts over all 128 partitions at
            # the full K=128 moving rate; the zero rows kill the other
            # head's Q contribution exactly.
            kt = qk_p.tile([128, N], f16, name=f"kt_{hh}", tag=f"kt{hh}")
            nc.sync.dma_start(out=kt, in_=kT[pair, i])
            kts[(pair, i)] = kt
            vt = vt_p.tile([128, N_MB, DV], bf16, name=f"vt_{hh}", tag=f"vt{hh}")
            nc.sync.dma_start(out=vt, in_=vp[hh])
            vts.append(vt)

    # One head-chunk at a time; S ring-buffered so S matmuls run ahead
    # of the exp stream, PV trails by LAG windows consuming pt tiles.
    for nh in range(2):
        for pair in range(2):
            for i, plo in ((0, 0), (1, 64)):
                qt, kt = qts[pair], kts[(pair, i)]
                hh = 2 * pair + i
                oacc = opool.tile([65, 2, 512], f32,
                                  name=f"o_{pair}_{nh}_{i}", tag="o")
                pts = {}
                # PV lags S by LAG m-blocks: S(mb) completes well before
                # exp(mb)'s window so its done-sem never stalls ACT; PV
                # fills the remaining PE time.
                LAG = 4
                for mb in range(N_MB + LAG):
                    if mb < N_MB:
                        msl = slice(mb * 128, (mb + 1) * 128)
                        sp = spool.tile([128, NH], f32,
                                        name=f"s_{pair}_{nh}_{i}_{mb}", tag="s")
                        for j in range(2):
                            jsl = slice(nh * NH + j * 512, nh * NH + (j + 1) * 512)
                            nc.tensor.matmul(
                                out=sp[:, j * 512:(j + 1) * 512],
                                lhsT=kt[:, msl],
                                rhs=qt[:, jsl],
                                start=True, stop=True)
                    if mb >= LAG:
                        pmb = mb - LAG
                        pt = pts.pop(pmb)
                        for j in range(2):
                            nc.tensor.matmul(
                                out=oacc[:, j, :],
                                lhsT=vts[hh][:, pmb, :],
                                rhs=pt[:, j * 512:(j + 1) * 512],
                                start=(pmb == 0),
                                stop=(pmb == N_MB - 1))
                    if mb < N_MB:
                        pt = ppool.tile([128, NH], bf16,
                                        name=f"pt_{pair}_{nh}_{i}_{mb}", tag="p")
                        nc.scalar.activation(pt, sp, EXP)
                        pts[mb] = pt

                # finalize: evacuate unnormalized O^T (+denominator row) and
                # DMA out; host normalizes+transposes during unshard.
                osb = finsb.tile([65, NH], f32,
                                 name=f"osb_{pair}_{nh}_{i}", tag=f"os{i}")
                nc.vector.tensor_copy(
                    osb, oacc.rearrange("p a b -> p (a b)"))
                nc.sync.dma_start(
                    out=out[hh][:, nh * NH:(nh + 1) * NH], in_=osb)


def build(repeat=1):
    nc = bacc.Bacc("TRN2", target_bir_lowering=False, debug=False)
    qT = nc.dram_tensor("qT", [2, 128, N], f16, kind="ExternalInput").ap()
    kT = nc.dram_tensor("kT", [2, 2, 128, N], f16, kind="ExternalInput").ap()
    vp = nc.dram_tensor("vp", [HEADS_PER_CORE, 128, N_MB, DV], bf16,
                        kind="ExternalInput").ap()
    out = nc.dram_tensor("out", [HEADS_PER_CORE, DV, N], f32,
                         kind="ExternalOutput").ap()

    from contextlib import ExitStack
    with tile.TileContext(nc) as tc, ExitStack() as ctx:
        qk_p = ctx.enter_context(tc.tile_pool(name="qk", bufs=2))
        vt_p = ctx.enter_context(tc.tile_pool(name="vt", bufs=2))
        spool = ctx.enter_context(tc.tile_pool(name="spool", bufs=3, space="PSUM"))
        ppool = ctx.enter_context(tc.tile_pool(name="ppool", bufs=14))
        opool = ctx.enter_context(tc.tile_pool(name="opool", bufs=1, space="PSUM"))
        finsb = ctx.enter_context(tc.tile_pool(name="finsb", bufs=2))

        pools = (qk_p, vt_p, spool, ppool, opool, finsb)

        if repeat == 1:
            emit_body(nc, qT, kT, vp, out, pools)
        else:
            with tc.For_i(0, repeat, 1, hint_engines=(
                    mybir.EngineType.PE, mybir.EngineType.Activation,
                    mybir.EngineType.DVE, mybir.EngineType.SP)):
                emit_body(nc, qT, kT, vp, out, pools)

    nc.compile()
    return nc


_NC_CACHE = {}


def _get_nc(repeat=1):
    if repeat not in _NC_CACHE:
        _NC_CACHE[repeat] = build(repeat)
    return _NC_CACHE[repeat]


def _to_bf16(x):
    """Round fp32 -> bf16 (round-to-nearest-even), return uint16 view."""
    u = x.view(np.uint32)
    rounded = (u + 0x7FFF + ((u >> 16) & 1)) >> 16
    return rounded.astype(np.uint16)


def run_sharded(query, key, value, repeat=1, **spmd_kwargs):
    """query/key/value: [B, N, H, D] fp32 -> out [B, H, N, D] fp32."""
    import ml_dtypes
    nc = _get_nc(repeat)
    # [B, N, H, D] -> [B*H, D, N] fp16 for Q/K; [B*H, N, D+1] bf16 for V'
    qt = np.ascontiguousarray(
        np.transpose(query, (0, 2, 3, 1))).reshape(B * H, D, N).astype(np.float16)
    kh = np.ascontiguousarray(
        np.transpose(key, (0, 2, 3, 1))).reshape(B * H, D, N).astype(np.float16)
    # zero-padded per-head K: head i's d-rows at partitions 64*(i%2)
    kt = np.zeros((B * H, 2, 64, N), dtype=np.float16)
    for h in range(B * H):
        kt[h, (h % 2)] = kh[h]
    vh = np.ascontiguousarray(np.transpose(value, (0, 2, 1, 3))).reshape(B * H, N, D)
    vp = np.empty((B * H, N, DV), dtype=np.uint16)
    vp[:, :, :D] = _to_bf16(vh)
    vp[:, :, D] = 0x3F80  # 1.0 in bf16
    # device tile layout [128, N_MB, DV]: partition p holds m-rows mb*128+p
    vp = np.ascontiguousarray(
        vp.reshape(B * H, N_MB, 128, DV).transpose(0, 2, 1, 3))
    vp = vp.view(ml_dtypes.bfloat16)
    in_maps = []
    for c in range(N_CORES):
        hs = slice(c * HEADS_PER_CORE, (c + 1) * HEADS_PER_CORE)
        in_maps.append({
            "qT": qt[hs].reshape(2, 128, N),
            "kT": kt[hs].reshape(2, 2, 128, N),
            "vp": vp[hs],
        })
    res = run_bass_kernel_spmd(nc, in_maps, core_ids=list(range(N_CORES)),
                               **spmd_kwargs)
    # [8, 4, 65, N]: rows 0..63 = unnormalized O^T, row 64 = denominator
    outs = np.stack([res.results[c]["out"] for c in range(N_CORES)])
    num = outs[:, :, :D, :]          # [8, 4, 64, N]
    den = outs[:, :, D:D + 1, :]     # [8, 4, 1, N]
    o = (num / den).transpose(0, 1, 3, 2)  # [8, 4, N, 64]
    return np.ascontiguousarray(o.reshape(B, H, N, D).astype(np.float32))


def kernel(query, key, value):
    query = np.asarray(query, dtype=np.float32)
    key = np.asarray(key, dtype=np.float32)
    value = np.asarray(value, dtype=np.float32)
    return run_sharded(query, key, value)


if __name__ == "__main__":
    rng = np.random.default_rng(0)
    q = rng.standard_normal((B, N, H, D), dtype=np.float32)
    k = rng.standard_normal((B, N, H, D), dtype=np.float32)
    v = rng.standard_normal((B, N, H, D), dtype=np.float32)
    o = kernel(q, k, v)
    print("out shape:", o.shape, o.dtype)


# revision 25
# speedup vs baseline: 1.0647x; 1.0647x over previous
"""Exact attention (B=2, N=2048, H=16, D=64, fp32) on 8 Trainium2 NeuronCores.

v6 design, built from HW microbenchmarks (mb.py). Measured facts driving it:
  - ACT exp runs at ~1.23 ns/elem (not the documented 1 cyc @1.2GHz), so
    exp over all N^2 elems is a ~170us/core floor: ACT paces the kernel.
  - Matmuls with K=64 contraction stream the moving operand at ~half the
    K=128 rate (~573 vs ~330 ns per 512-col MM) regardless of dtype, and
    every matmul pays its LDWEIGHTS serially. The old baseline (f32r S +
    P^T-stationary PV with 128-col LDW churn) was PE-bound at ~272us busy.
  - Fixes here: Q/K in fp16, with K zero-padded per head to a full 128
    partitions (the zero rows contribute exactly 0, so the S matmul runs
    at the K=128 moving rate: ~84us instead of ~147us), PV in O^T
    orientation with V' = [V | 1] bf16 stationary (65-col LDW, moving P^T
    at K=128 rate: ~85us), deep pt buffering (bufs=14) so ACT runs far
    ahead of the PV consumers. PE ends up fully hidden under ACT.
  - Finalize: device returns unnormalized O^T [65, N] per head (row 64 =
    softmax denominator from the V' ones column) straight to DRAM; host
    divides and transposes during unshard (O(N*d) postprocess, same bytes
    DMA'd as the normalized [N, 64] output would be).

Sharding: 32 (batch, head) pairs across 8 cores, 4 heads per core; Q^T/K^T
stored packed 2-heads-per-128-partitions (d=64 rows each), heads processed
one chunk at a time.

Per-core schedule, per (n-chunk of 1024, pair, head), m-blocks of 128:
  PSUM: S ring of 3 [128,1024] f32 (6 banks) + one OT [65,2,512] (2 banks).
  per mb: S(mb) [2 MMs], PV(mb-LAG) [2 MMs accumulating into OT],
  exp(mb) [one ACT instr, S->pt bf16]. S runs ahead inside the exp
  window so the S->exp handoff semaphore never lands on the ACT critical
  path; PV lags by LAG windows and fills PE gaps.
  Chunk end: DVE evacuates OT to SBUF fp32, DMA out [65, 1024]; the next
  chunk's first PV (start=True) waits on that evac (Tile serializes the
  PSUM bank reuse), which is hidden since it happens LAG windows in.
"""

import os
import sys

os.environ.setdefault("MYCRO_LOCAL_CACHE", "1")
sys.path.insert(0, "/opt/trn_rl_repo")

import numpy as np

import concourse.bacc as bacc
import concourse.mybir as mybir
import concourse.tile as tile
from concourse.bass_utils import run_bass_kernel_spmd

f32 = mybir.dt.float32
f16 = mybir.dt.float16
bf16 = mybir.dt.bfloat16

B, N, H, D = 2, 2048, 16, 64
HEADS_PER_CORE = 4
N_CORES = 8
NH = 1024          # n-chunk width
N_MB = N // 128    # 16 m-blocks of 128 rows
DV = D + 1         # V plus ones column
EXP = mybir.ActivationFunctionType.Exp


def emit_body(nc, qT, kT, vp, out, pools):
    qk_p, vt_p, spool, ppool, opool, finsb = pools

    # --- input DMAs ---
    qts, kts, vts = [], {}, []
    for pair in range(2):
        qt = qk_p.tile([128, N], f16, name=f"qt_{pair}", tag=f"qt{pair}")
        nc.sync.dma_start(out=qt, in_=qT[pair])
        qts.append(qt)
        for i in range(2):
            hh = 2 * pair + i
            # K for head i zero-padded to 128 rows (data at rows 64i..64i+63,
            # zeros elsewhere): S matmul contracts over all 128 partitions at
            # the full K=128 moving rate; the zero rows kill the other
            # head's Q contribution exactly.
            kt = qk_p.tile([128, N], f16, name=f"kt_{hh}", tag=f"kt{hh}")
            nc.sync.dma_start(out=kt, in_=kT[pair, i])
            kts[(pair, i)] = kt
            vt = vt_p.tile([128, N_MB, DV], bf16, name=f"vt_{hh}", tag=f"vt{hh}")
            nc.sync.dma_start(out=vt, in_=vp[hh])
            vts.append(vt)

    # One head-chunk at a time; S ring-buffered so S matmuls run ahead
    # of the exp stream, PV trails by LAG windows consuming pt tiles.
    for nh in range(2):
        for pair in range(2):
            for i, plo in ((0, 0), (1, 64)):
                qt, kt = qts[pair], kts[(pair, i)]
                hh = 2 * pair + i
                oacc = opool.tile([65, 2, 512], f32,
                                  name=f"o_{pair}_{nh}_{i}", tag="o")
                pts = {}
                # PV lags S by LAG m-blocks: S(mb) completes well before
                # exp(mb)'s window so its done-sem never stalls ACT; PV
                # fills the remaining PE time.
                LAG = 3
                for mb in range(N_MB + LAG):
                    if mb < N_MB:
                        msl = slice(mb * 128, (mb + 1) * 128)
                        sp = spool.tile([128, NH], f32,
                                        name=f"s_{pair}_{nh}_{i}_{mb}", tag="s")
                        for j in range(2):
                            jsl = slice(nh * NH + j * 512, nh * NH + (j + 1) * 512)
                            nc.tensor.matmul(
                                out=sp[:, j * 512:(j + 1) * 512],
                                lhsT=kt[:, msl],
                                rhs=qt[:, jsl],
                                start=True, stop=True)
                    if mb >= LAG:
                        pmb = mb - LAG
                        pt = pts.pop(pmb)
                        for j in range(2):
                            nc.tensor.matmul(
                                out=oacc[:, j, :],
                                lhsT=vts[hh][:, pmb, :],
                                rhs=pt[:, j * 512:(j + 1) * 512],
                                start=(pmb == 0),
                                stop=(pmb == N_MB - 1))
                    if mb < N_MB:
                        pt = ppool.tile([128, NH], bf16,
                                        name=f"pt_{pair}_{nh}_{i}_{mb}", tag="p")
                        nc.scalar.activation(pt, sp, EXP)
                        pts[mb] = pt

                # finalize: evacuate unnormalized O^T (+denominator row) and
                # DMA out; host normalizes+transposes during unshard.
                osb = finsb.tile([65, NH], f32,
                                 name=f"osb_{pair}_{nh}_{i}", tag=f"os{i}")
                nc.vector.tensor_copy(
                    osb, oacc.rearrange("p a b -> p (a b)"))
                nc.sync.dma_start(
                    out=out[hh][:, nh * NH:(nh + 1) * NH], in_=osb)


def build(repeat=1):
    nc = bacc.Bacc("TRN2", target_bir_lowering=False, debug=False)
    qT = nc.dram_tensor("qT", [2, 128, N], f16, kind="ExternalInput").ap()
    kT = nc.dram_tensor("kT", [2, 2, 128, N], f16, kind="ExternalInput").ap()
    vp = nc.dram_tensor("vp", [HEADS_PER_CORE, 128, N_MB, DV], bf16,
                        kind="ExternalInput").ap()
    out = nc.dram_tensor("out", [HEADS_PER_CORE, DV, N], f32,
                         kind="ExternalOutput").ap()

    from contextlib import ExitStack
    with tile.TileContext(nc) as tc, ExitStack() as ctx:
        qk_p = ctx.enter_context(tc.tile_pool(name="qk", bufs=2))
        vt_p = ctx.enter_context(tc.tile_pool(name="vt", bufs=2))
        spool = ctx.enter_context(tc.tile_pool(name="spool", bufs=3, space="PSUM"))
        ppool = ctx.enter_context(tc.tile_pool(name="ppool", bufs=14))
        opool = ctx.enter_context(tc.tile_pool(name="opool", bufs=1, space="PSUM"))
        finsb = ctx.enter_context(tc.tile_pool(name="finsb", bufs=2))

        pools = (qk_p, vt_p, spool, ppool, opool, finsb)

        if repeat == 1:
            emit_body(nc, qT, kT, vp, out, pools)
        else:
            with tc.For_i(0, repeat, 1, hint_engines=(
                    mybir.EngineType.PE, mybir.EngineType.Activation,
                    mybir.EngineType.DVE, mybir.EngineType.SP)):
                emit_body(nc, qT, kT, vp, out, pools)

    nc.compile()
    return nc


_NC_CACHE = {}


def _get_nc(repeat=1):
    if repeat not in _NC_CACHE:
        _NC_CACHE[repeat] = build(repeat)
    return _NC_CACHE[repeat]


def _to_bf16(x):
    """Round fp32 -> bf16 (round-to-nearest-even), return uint16 view."""
    u = x.view(np.uint32)
    rounded = (u + 0x7FFF + ((u >> 16) & 1)) >> 16
    return rounded.astype(np.uint16)


def run_sharded(query, key, value, repeat=1, **spmd_kwargs):
    """query/key/value: [B, N, H, D] fp32 -> out [B, H, N, D] fp32."""
    import ml_dtypes
    nc = _get_nc(repeat)
    # [B, N, H, D] -> [B*H, D, N] fp16 for Q/K; [B*H, N, D+1] bf16 for V'
    qt = np.ascontiguousarray(
        np.transpose(query, (0, 2, 3, 1))).reshape(B * H, D, N).astype(np.float16)
    kh = np.ascontiguousarray(
        np.transpose(key, (0, 2, 3, 1))).reshape(B * H, D, N).astype(np.float16)
    # zero-padded per-head K: head i's d-rows at partitions 64*(i%2)
    kt = np.zeros((B * H, 2, 64, N), dtype=np.float16)
    for h in range(B * H):
        kt[h, (h % 2)] = kh[h]
    vh = np.ascontiguousarray(np.transpose(value, (0, 2, 1, 3))).reshape(B * H, N, D)
    vp = np.empty((B * H, N, DV), dtype=np.uint16)
    vp[:, :, :D] = _to_bf16(vh)
    vp[:, :, D] = 0x3F80  # 1.0 in bf16
    # device tile layout [128, N_MB, DV]: partition p holds m-rows mb*128+p
    vp = np.ascontiguousarray(
        vp.reshape(B * H, N_MB, 128, DV).transpose(0, 2, 1, 3))
    vp = vp.view(ml_dtypes.bfloat16)
    in_maps = []
    for c in range(N_CORES):
        hs = slice(c * HEADS_PER_CORE, (c + 1) * HEADS_PER_CORE)
        in_maps.append({
            "qT": qt[hs].reshape(2, 128, N),
            "kT": kt[hs].reshape(2, 2, 128, N),
            "vp": vp[hs],
        })
    res = run_bass_kernel_spmd(nc, in_maps, core_ids=list(range(N_CORES)),
                               **spmd_kwargs)
    # [8, 4, 65, N]: rows 0..63 = unnormalized O^T, row 64 = denominator
    outs = np.stack([res.results[c]["out"] for c in range(N_CORES)])
    num = outs[:, :, :D, :]          # [8, 4, 64, N]
    den = outs[:, :, D:D + 1, :]     # [8, 4, 1, N]
    o = (num / den).transpose(0, 1, 3, 2)  # [8, 4, N, 64]
    return np.ascontiguousarray(o.reshape(B, H, N, D).astype(np.float32))


def kernel(query, key, value):
    query = np.asarray(query, dtype=np.float32)
    key = np.asarray(key, dtype=np.float32)
    value = np.asarray(value, dtype=np.float32)
    return run_sharded(query, key, value)


if __name__ == "__main__":
    rng = np.random.default_rng(0)
    q = rng.standard_normal((B, N, H, D), dtype=np.float32)
    k = rng.standard_normal((B, N, H, D), dtype=np.float32)
    v = rng.standard_normal((B, N, H, D), dtype=np.float32)
    o = kernel(q, k, v)
    print("out shape:", o.shape, o.dtype)
